# revision 30
# baseline (speedup 1.0000x reference)
"""Trainium2 Bass kernel for nn_DAWNLayer (moe_routing).

Strategy: data-parallel over batch B=8 across the 8 NeuronCores; each core
runs the full layer for one [S=1024, D=768] slice. Weights are replicated
and pre-processed once on the host (cached):
  * LN gains g1/g2 folded into the consumer weight matrices (q_w, k_w,
    basis_A, neuron table, up_w); zero biases compiled out.
  * Weights pre-transposed into SBUF layout and pre-cast: fp8 e4m3 for
    Q/K/AO projections, basis_A and the neuron table (DoubleRow matmuls),
    bf16 for the FFN and V-out.
  * neuron table = (softmax(recipe) * sigmoid(ctx_pat.sum/S)) @ basis_emb,
    exploiting that attn.mean(-1) of a softmax is the constant 1/S.

Device-side structure (per core):
  A: LN1 -> z (bf16) -> PE-transpose -> norm8 (fp8, d-major)
  B: routing scores (fp8 DR matmul), top-8 masked softmax, maskT,
     token_recipe
  C: basis projection (fp8 DR) * token_recipe -> v_sem -> V8 (fp8, with an
     appended ones column per head)
  D: Q/K projections (fp8 DR) -> QT/KT (fp8, head-major)
  E: attention without transposes: scoresT[kv,q] = KT-stationary matmul,
     exp -> fp8 attnT (unnormalized, shift -1.5), AV via fp8 DoubleRow with
     the ones column yielding the softmax denominator; normalize after AV.
  F: AO projection (fp8 DR) + residual (ao_b folded into x2 on host)
  G: LN2 -> n2T bf16 (g2/b2 folded into up weights)
  H: FFN (bf16), residual, writeback.
"""

import os
import numpy as np

B, S, D = 8, 1024, 768
H, DH = 12, 64
DFF = 3072
NN, NB, RK, TOPK = 96, 32, 64, 8
P = 128
TILES = S // P          # 8 token tiles
DC = D // P             # 6 chunks of d-model
FC = DFF // P           # 24 chunks of d_ff
NCORES = 8
HW = 65                 # head width in V (DH + ones column)
HPAD = 784              # TILES stride in V8 (pair-dim stride must be %16==0)

_CACHE = {}


def _f8(x):
    import ml_dtypes
    return np.clip(np.asarray(x, np.float32), -240.0, 240.0).astype(
        ml_dtypes.float8_e4m3)


def _bf(x):
    import ml_dtypes
    return np.asarray(x, np.float32).astype(ml_dtypes.bfloat16)


def _pon(w):
    """[D_in, N] -> [P, DC_in, N]  ((o p) n -> p o n)."""
    din, n = w.shape
    return np.ascontiguousarray(
        w.reshape(din // P, P, n).transpose(1, 0, 2))


def _prep(inputs):
    """Host-side one-time weight prep -> dict of DMA-ready arrays + flags."""
    f32 = np.float32
    g1 = np.asarray(inputs["n1_g"], f32)
    b1 = np.asarray(inputs["n1_b"], f32)
    g2 = np.asarray(inputs["n2_g"], f32)
    b2 = np.asarray(inputs["n2_b"], f32)
    q_w = np.asarray(inputs["q_w"], f32)
    k_w = np.asarray(inputs["k_w"], f32)
    ao_w = np.asarray(inputs["ao_w"], f32)
    up_w = np.asarray(inputs["up_w"], f32)
    down_w = np.asarray(inputs["down_w"], f32)
    basis_A = np.asarray(inputs["basis_A"], f32)
    vout_w = np.asarray(inputs["vout_w"], f32)
    recipe = np.asarray(inputs["recipe"], f32)
    ctx = np.asarray(inputs["ctx_pat"], f32)
    bemb = np.asarray(inputs["basis_emb"], f32)

    r = recipe - recipe.max(-1, keepdims=True)
    e = np.exp(r)
    recipe_norm = e / e.sum(-1, keepdims=True)                  # [NN, NB]
    sig = 1.0 / (1.0 + np.exp(-ctx.sum(-1) / S))                # [NN]
    nemb_s = (recipe_norm * sig[:, None]) @ bemb                # [NN, D]

    w = {}
    w["qw8"] = _f8(_pon(q_w * g1[:, None]))                     # [P,DC,D]
    w["kw8"] = _f8(_pon(k_w * g1[:, None]))
    w["aow8"] = _f8(_pon(ao_w))
    # basis8[p, o, n*RK + r] = basis_A[n, o*P+p, r] * g1
    bA = (basis_A * g1[None, :, None]).transpose(1, 0, 2).reshape(D, NB * RK)
    w["basis8"] = _f8(_pon(bA))                                 # [P,DC,NB*RK]
    w["upw"] = _f8(_pon(up_w * g2[:, None]))                    # [P,DC,DFF]
    w["downw"] = _bf(_pon(down_w))                              # [P,FC,D]
    w["voutw"] = _bf(vout_w)                                    # [RK,D]
    # nembT[p, o, n] = nemb_s[n, o*P+p] * g1[o*P+p]
    w["nembT"] = _f8(_pon((nemb_s * g1[None, :]).T))            # [P,DC,NN]
    w["recipe_norm"] = _bf(recipe_norm)                         # [NN,NB]

    flags = {}
    qb = np.asarray(inputs["q_b"], f32) + b1 @ q_w
    kb = np.asarray(inputs["k_b"], f32) + b1 @ k_w
    flags["qb"] = bool(np.any(qb)); flags["kb"] = bool(np.any(kb))
    if flags["qb"]:
        w["qb"] = np.ascontiguousarray(qb.reshape(DC, P).T).astype(f32)
    if flags["kb"]:
        w["kb"] = np.ascontiguousarray(kb.reshape(DC, P).T).astype(f32)
    finb = b1 @ nemb_s.T                                        # [NN]
    flags["finb"] = bool(np.any(finb))
    if flags["finb"]:
        w["finb"] = np.ascontiguousarray(
            np.broadcast_to(finb, (P, NN))).astype(f32)
    cb1 = np.einsum("d,ndr->nr", b1, basis_A).reshape(NB * RK)
    flags["cb1"] = bool(np.any(cb1))
    if flags["cb1"]:
        w["cb1"] = _f8(cb1.reshape(1, NB * RK))
    vob = np.asarray(inputs["vout_b"], f32)
    flags["voutb"] = bool(np.any(vob))
    if flags["voutb"]:
        w["voutb"] = np.ascontiguousarray(np.broadcast_to(vob, (P, D))).astype(f32)
    upb = np.asarray(inputs["up_b"], f32) + b2 @ up_w
    flags["upb"] = bool(np.any(upb))
    if flags["upb"]:
        w["upb"] = np.ascontiguousarray(upb.reshape(FC, P).T).astype(f32)
    dnb = np.asarray(inputs["down_b"], f32)
    flags["downb"] = bool(np.any(dnb))
    if flags["downb"]:
        w["downb"] = np.ascontiguousarray(np.broadcast_to(dnb, (P, D))).astype(f32)
    aob = np.asarray(inputs["ao_b"], f32)
    return w, flags, aob


def _build(flags):
    import concourse.bass as bass
    import concourse.bacc as bacc
    import concourse.mybir as mybir
    from concourse.tile import TileContext
    from concourse.masks import make_identity
    from contextlib import ExitStack

    f32 = mybir.dt.float32
    bf16 = mybir.dt.bfloat16
    f8 = mybir.dt.float8e4
    AF = mybir.ActivationFunctionType
    OP = mybir.AluOpType
    AX = mybir.AxisListType
    DR = mybir.MatmulPerfMode.DoubleRow

    nc = bacc.Bacc("TRN2", target_bir_lowering=False, debug=False,
                   num_devices=NCORES)

    d_in = {}
    def din(name, shape, dt):
        d_in[name] = nc.dram_tensor(name, list(shape), dt, kind="ExternalInput")
        return d_in[name]

    x_d = din("x", (S, D), f32)
    x2_d = din("x2", (S, D), f32)
    qw8_d = din("qw8", (P, DC, D), f8)
    kw8_d = din("kw8", (P, DC, D), f8)
    aow8_d = din("aow8", (P, DC, D), f8)
    basis8_d = din("basis8", (P, DC, NB * RK), f8)
    upw_d = din("upw", (P, DC, DFF), f8)
    downw_d = din("downw", (P, FC, D), bf16)
    voutw_d = din("voutw", (RK, D), bf16)
    nembT_d = din("nembT", (P, DC, NN), f8)
    recipe_d = din("recipe_norm", (NN, NB), bf16)
    if flags["qb"]:
        qb_d = din("qb", (P, DC), f32)
    if flags["kb"]:
        kb_d = din("kb", (P, DC), f32)
    if flags["finb"]:
        finb_d = din("finb", (P, NN), f32)
    if flags["cb1"]:
        cb1_d = din("cb1", (1, NB * RK), f8)
    if flags["voutb"]:
        voutb_d = din("voutb", (P, D), f32)
    if flags["upb"]:
        upb_d = din("upb", (P, FC), f32)
    if flags["downb"]:
        downb_d = din("downb", (P, D), f32)
    y_d = nc.dram_tensor("y", [S, D], f32, kind="ExternalOutput")

    with TileContext(nc, pool_alloc_mode="queue") as tc, ExitStack() as top, \
         nc.allow_low_precision(reason="bf16/fp8 pipeline, fp32 accum"):
        psA = top.enter_context(tc.tile_pool(name="psA", bufs=2, space="PSUM"))
        psB = top.enter_context(tc.tile_pool(name="psB", bufs=2, space="PSUM"))
        psT = top.enter_context(tc.tile_pool(name="psT", bufs=2, space="PSUM"))
        singles = top.enter_context(tc.tile_pool(name="singles", side="left", bufs=1))
        work = top.enter_context(tc.tile_pool(name="work", side="left", bufs=2))
        xload = top.enter_context(tc.tile_pool(name="xload", side="left", bufs=3))

        # left-side pools, bottom (longest-lived) to top (shortest-lived)
        es_dn = ExitStack()
        p_dn = es_dn.enter_context(tc.tile_pool(name="p_dn", side="left", bufs=1))
        es_up = ExitStack()
        p_up = es_up.enter_context(tc.tile_pool(name="p_up", side="left", bufs=1))
        es_x1 = ExitStack()
        p_x1 = es_x1.enter_context(tc.tile_pool(name="p_x1", side="left", bufs=1))
        es_ao = ExitStack()
        p_ao = es_ao.enter_context(tc.tile_pool(name="p_ao", side="left", bufs=1))
        es_v = ExitStack()
        p_v = es_v.enter_context(tc.tile_pool(name="p_v", side="left", bufs=1))
        es_at = ExitStack()
        p_at = es_at.enter_context(tc.tile_pool(name="p_at", side="left", bufs=2))
        es_n1 = ExitStack()
        p_n1 = es_n1.enter_context(tc.tile_pool(name="p_n1", side="left", bufs=1))
        es_b = ExitStack()
        p_b = es_b.enter_context(tc.tile_pool(name="p_b", side="left", bufs=1))
        # right-side pools
        es_n2 = ExitStack()
        p_n2 = es_n2.enter_context(tc.tile_pool(name="p_n2", side="right", bufs=1))
        es_qt = ExitStack()
        p_qt = es_qt.enter_context(tc.tile_pool(name="p_qt", side="right", bufs=1))
        es_w1 = ExitStack()
        p_w1 = es_w1.enter_context(tc.tile_pool(name="p_w1", side="right", bufs=1))

        def pA():  return psA.tile([P, 1024], f32, tag="psA", name="psA_t")
        def pB():  return psB.tile([P, 512], f32, tag="psB", name="psB_t")

        # ---- constants ----
        id_b = singles.tile([P, P], bf16)
        make_identity(nc, id_b)
        eps_t = singles.tile([P, 1], f32)
        nc.vector.memset(eps_t, 1e-5)
        shift_t = singles.tile([P, 1], f32)
        nc.vector.memset(shift_t, -1.5)
        ones_bf = singles.tile([1, RK], bf16)
        nc.vector.memset(ones_bf, 1.0)

        # ---- weights (straight DMA, pre-laid-out) ----
        nembT = singles.tile([P, DC, NN], f8)
        nc.scalar.dma_start(nembT, nembT_d.ap())
        recipe_sb = singles.tile([NN, NB], bf16)
        nc.scalar.dma_start(recipe_sb, recipe_d.ap())
        basis8 = p_w1.tile([P, DC, NB * RK], f8)
        nc.scalar.dma_start(basis8, basis8_d.ap())
        qw8 = p_w1.tile([P, DC, D], f8)
        nc.scalar.dma_start(qw8, qw8_d.ap())
        kw8 = p_w1.tile([P, DC, D], f8)
        nc.scalar.dma_start(kw8, kw8_d.ap())
        voutw = singles.tile([RK, D], bf16)
        nc.scalar.dma_start(voutw, voutw_d.ap())
        aow8 = singles.tile([P, DC, D], f8)
        nc.gpsimd.dma_start(aow8, aow8_d.ap())
        if flags["qb"]:
            qb_sb = singles.tile([P, DC], f32)
            nc.scalar.dma_start(qb_sb, qb_d.ap())
        if flags["kb"]:
            kb_sb = singles.tile([P, DC], f32)
            nc.scalar.dma_start(kb_sb, kb_d.ap())
        if flags["finb"]:
            finb_sb = singles.tile([P, NN], f32)
            nc.scalar.dma_start(finb_sb, finb_d.ap())
        if flags["cb1"]:
            cb1_sb = singles.tile([1, NB * RK], f8)
            nc.scalar.dma_start(cb1_sb, cb1_d.ap())
            ones18 = singles.tile([1, P], f8)
            nc.vector.memset(ones18, 1.0)
        if flags["voutb"]:
            voutb_sb = singles.tile([P, D], f32)
            nc.scalar.dma_start(voutb_sb, voutb_d.ap())
        if flags["upb"]:
            upb_sb = singles.tile([P, FC], f32)
            nc.scalar.dma_start(upb_sb, upb_d.ap())
        if flags["downb"]:
            downb_sb = singles.tile([P, D], f32)
            nc.scalar.dma_start(downb_sb, downb_d.ap())

        # ---- Phase A: LN1 -> norm8 (fp8, d-major) ----
        norm8 = p_n1.tile([P, DC, S], f8, tag="norm8")

        def layernorm_z(xt, out_z):
            """z = (x - mean) * rstd, token-major; out_z bf16 [P, D]."""
            stats = work.tile([P, 3, 6], f32, tag="ln_stats")
            xv = xt.rearrange("p (a q) -> p a q", a=3)
            for a in range(3):
                nc.vector.bn_stats(out=stats[:, a, :], in_=xv[:, a, :])
            mv = work.tile([P, 2], f32, tag="ln_mv")
            nc.vector.bn_aggr(out=mv, in_=stats)
            std = work.tile([P, 1], f32, tag="ln_std")
            nc.scalar.activation(out=std, in_=mv[:, 1:2], func=AF.Sqrt,
                                 bias=eps_t[:, 0:1])
            rstd = work.tile([P, 1], f32, tag="ln_rstd")
            nc.vector.reciprocal(rstd, std)
            nc.vector.tensor_scalar(out=out_z, in0=xt, scalar1=mv[:, 0:1],
                                    scalar2=rstd[:, 0:1], op0=OP.subtract,
                                    op1=OP.mult)

        x_re = x_d.ap().rearrange("(t p) d -> p t d", p=P)
        for t in range(TILES):
            xt = xload.tile([P, D], f32, tag="xt")
            nc.sync.dma_start(xt, x_re[:, t, :])
            z = work.tile([P, D], bf16, tag="zt")
            layernorm_z(xt, z)
            for g in range(2):
                ps = psT.tile([P, 3 * P], bf16, tag="psT", name="psT_t")
                for j in range(3):
                    c = g * 3 + j
                    nc.tensor.transpose(ps[:, j * P:(j + 1) * P],
                                        z[:, c * P:(c + 1) * P], id_b)
                nc.scalar.activation(
                    out=norm8[:, g * 3:(g + 1) * 3, t * P:(t + 1) * P],
                    in_=ps.rearrange("p (j q) -> p j q", j=3), func=AF.Copy)

        # ---- Phase B: routing scores -> maskT, token_recipe ----
        maskT = p_b.tile([NN, S], bf16)
        tr_sb = singles.tile([P, TILES, NB], bf16)
        for t in range(TILES):
            ps = psT.tile([P, NN], f32, tag="psT", name="psT_t")
            for j in range(DC // 2):
                nc.tensor.matmul(ps[:P, :NN],
                                 lhsT=norm8[:, 2 * j:2 * j + 2,
                                            t * P:(t + 1) * P],
                                 rhs=nembT[:, 2 * j:2 * j + 2, :],
                                 start=(j == 0), stop=(j == DC // 2 - 1),
                                 perf_mode=DR)
            fin = work.tile([P, NN], f32, tag="fin")
            if flags["finb"]:
                nc.vector.tensor_tensor(out=fin, in0=ps[:P, :NN],
                                        in1=finb_sb, op=OP.add)
            else:
                nc.vector.tensor_copy(out=fin, in_=ps[:P, :NN])
            mx = work.tile([P, 8], f32, tag="mx")
            nc.vector.max(out=mx, in_=fin)
            nmx = work.tile([P, 1], f32, tag="nmx")
            nc.vector.tensor_scalar_mul(nmx, mx[:, 0:1], -1.0)
            e = work.tile([P, NN], bf16, tag="e")
            nc.scalar.activation(out=e, in_=fin, func=AF.Exp, bias=nmx[:, 0:1])
            msk = work.tile([P, NN], f32, tag="msk")
            nc.vector.tensor_scalar(out=msk, in0=fin, scalar1=mx[:, 7:8],
                                    scalar2=None, op0=OP.is_ge)
            nc.vector.tensor_tensor(out=e, in0=e, in1=msk, op=OP.mult)
            den = work.tile([P, 1], f32, tag="den")
            nc.vector.tensor_reduce(out=den, in_=e, axis=AX.X, op=OP.add)
            idn = work.tile([P, 1], f32, tag="idn")
            nc.vector.reciprocal(idn, den)
            nc.vector.tensor_scalar_mul(e, e, idn[:, 0:1])
            ps2 = psT.tile([P, P], bf16, tag="psT", name="psT_t")
            nc.tensor.transpose(ps2[:NN, :P], e[:, :NN], id_b)
            nc.vector.tensor_copy(out=maskT[:, t * P:(t + 1) * P],
                                  in_=ps2[:NN, :P])
            ps3 = psT.tile([P, NB], f32, tag="psT", name="psT_t")
            nc.tensor.matmul(ps3[:P, :NB], lhsT=maskT[:, t * P:(t + 1) * P],
                             rhs=recipe_sb, start=True, stop=True)
            nc.vector.tensor_copy(out=tr_sb[:, t, :], in_=ps3[:P, :NB])

        # ---- Phase C: basis projection (fp8 DR) -> v_sem -> V8 ----
        vsemT = singles.tile([RK, TILES, P], bf16)
        NGRP = 4
        GN = NB // NGRP     # 8 basis entries per 512-wide group
        for t in range(TILES):
            scf = work.tile([P, NB, RK], bf16, tag="scf")
            for g in range(NGRP):
                ps = pB()
                first = True
                if flags["cb1"]:
                    nc.tensor.matmul(
                        ps[:, :512], lhsT=ones18,
                        rhs=cb1_sb[:, g * 512:(g + 1) * 512],
                        start=True, stop=False, skip_group_check=True)
                    first = False
                for j in range(DC // 2):
                    nc.tensor.matmul(
                        ps[:, :512],
                        lhsT=norm8[:, 2 * j:2 * j + 2, t * P:(t + 1) * P],
                        rhs=basis8[:, 2 * j:2 * j + 2, g * 512:(g + 1) * 512],
                        start=first and (j == 0), stop=(j == DC // 2 - 1),
                        perf_mode=DR)
                sc = scf[:, g * GN:(g + 1) * GN, :]
                nc.scalar.activation(out=sc, in_=ps.rearrange(
                    "p (n r) -> p n r", n=GN), func=AF.Copy)
                nc.vector.tensor_tensor(
                    out=sc, in0=sc,
                    in1=tr_sb[:, t, g * GN:(g + 1) * GN, None].to_broadcast(
                        [P, GN, RK]),
                    op=OP.mult)
            nc.vector.tensor_tensor(out=scf[:, 0:16, :], in0=scf[:, 0:16, :],
                                    in1=scf[:, 16:32, :], op=OP.add)
            nc.vector.tensor_tensor(out=scf[:, 0:8, :], in0=scf[:, 0:8, :],
                                    in1=scf[:, 8:16, :], op=OP.add)
            nc.vector.tensor_tensor(out=scf[:, 0:4, :], in0=scf[:, 0:4, :],
                                    in1=scf[:, 4:8, :], op=OP.add)
            nc.vector.tensor_tensor(out=scf[:, 0:2, :], in0=scf[:, 0:2, :],
                                    in1=scf[:, 2:4, :], op=OP.add)
            vsem = work.tile([P, RK], bf16, tag="vsem")
            nc.vector.tensor_tensor(out=vsem, in0=scf[:, 0, :],
                                    in1=scf[:, 1, :], op=OP.add)
            ps2 = psT.tile([P, P], bf16, tag="psT", name="psT_t")
            nc.tensor.transpose(ps2[:RK, :P], vsem, id_b)
            nc.vector.tensor_copy(out=vsemT[:, t, :], in_=ps2[:RK, :P])

        # ---- V build: V8 [P, TILES, H*HW] fp8 with ones columns ----
        V8 = p_v.tile([P, TILES, HPAD], f8, tag="V8")
        nc.vector.memset(V8, 1.0)
        for t in range(TILES):
            ps = pA()
            nc.tensor.matmul(ps[:, 0:512], lhsT=vsemT[:, t, :],
                             rhs=voutw[:, 0:512], start=True, stop=True)
            nc.tensor.matmul(ps[:, 512:768], lhsT=vsemT[:, t, :],
                             rhs=voutw[:, 512:768], start=True, stop=True)
            dst = V8[:, t, 0:H * HW].rearrange("p (h e) -> p h e", h=H)[:, :, 0:DH]
            src = ps[:, :768].rearrange("p (h e) -> p h e", h=H)
            if flags["voutb"]:
                nc.vector.tensor_tensor(
                    out=dst, in0=src,
                    in1=voutb_sb.rearrange("p (h e) -> p h e", h=H),
                    op=OP.add)
            else:
                nc.vector.tensor_copy(out=dst, in_=src)

        # ---- Phase D: Q/K projections (fp8 DR) -> QT/KT fp8 ----
        QT = p_qt.tile([P, DC, S], f8, tag="QT")
        KT = p_qt.tile([P, DC, S], f8, tag="KT")
        for (w8, bflag, bias_sb, out_t) in (
                (qw8, flags["qb"], (qb_sb if flags["qb"] else None), QT),
                (kw8, flags["kb"], (kb_sb if flags["kb"] else None), KT)):
            for m in range(DC):
                for half in range(2):
                    sl = slice(half * 512, (half + 1) * 512)
                    ps = pB()
                    for j in range(DC // 2):
                        nc.tensor.matmul(
                            ps[:, :512],
                            lhsT=w8[:, 2 * j:2 * j + 2, m * P:(m + 1) * P],
                            rhs=norm8[:, 2 * j:2 * j + 2, sl],
                            start=(j == 0), stop=(j == DC // 2 - 1),
                            perf_mode=DR)
                    if bflag:
                        nc.scalar.activation(out=out_t[:, m, sl], in_=ps,
                                             func=AF.Identity,
                                             bias=bias_sb[:, m:m + 1])
                    else:
                        nc.scalar.activation(out=out_t[:, m, sl], in_=ps,
                                             func=AF.Copy)


        es_b.close()
        es_n1.close()
        es_w1.close()

        # ---- FFN weight prefetch (overlaps attention) ----
        upw = p_up.tile([P, DC, DFF], f8, tag="upw")
        for c in range(DC):
            nc.scalar.dma_start(upw[:, c, :], upw_d.ap()[:, c, :])
        downw = p_dn.tile([P, FC, D], bf16, tag="downw")
        for c4 in range(FC // 4):
            nc.scalar.dma_start(downw[:, 4 * c4:4 * c4 + 4, :],
                                downw_d.ap()[:, 4 * c4:4 * c4 + 4, :])

        # ---- Phase E: attention (AV of head h-1 overlaps exp of head h) ----
        aoutT = p_ao.tile([P, DC, S], f8, tag="aoutT")

        def emit_av(h, attnT):
            hp = (h % 2) * DH
            hc = h // 2
            for qc in range(2):
                sl = slice(qc * 512, (qc + 1) * 512)
                psv = pB()
                for jt in range(TILES // 2):
                    nc.tensor.matmul(
                        psv[0:HW, :512],
                        lhsT=V8[:, 2 * jt:2 * jt + 2, h * HW:(h + 1) * HW],
                        rhs=attnT[:, 2 * jt:2 * jt + 2, sl],
                        start=(jt == 0), stop=(jt == TILES // 2 - 1),
                        perf_mode=DR)
                rden = work.tile([1, 512], bf16, tag="rden")
                nc.vector.reciprocal(rden, psv[DH:HW, :512])
                psd = psT.tile([RK, 512], f32, tag="psT", name="psT_t")
                nc.tensor.matmul(psd, lhsT=ones_bf, rhs=rden,
                                 start=True, stop=True)
                den64 = work.tile([RK, 512], bf16, tag="den64")
                nc.vector.tensor_copy(out=den64, in_=psd)
                nc.vector.tensor_tensor(
                    out=aoutT[hp:hp + DH, hc, sl], in0=psv[0:DH, :512],
                    in1=den64, op=OP.mult)

        prev = None
        for h in range(H):
            hp = (h % 2) * DH
            hc = h // 2
            attnT = p_at.tile([P, TILES, S], f8, tag="attnT")
            for kvc in range(TILES):
                ps = pA()
                for qh in range(2):
                    qsl = slice(qh * 512, (qh + 1) * 512)
                    nc.tensor.matmul(
                        ps[:, qsl],
                        lhsT=KT[hp:hp + DH, hc, kvc * P:(kvc + 1) * P],
                        rhs=QT[hp:hp + DH, hc, qsl],
                        start=True, stop=True)
                nc.scalar.activation(out=attnT[:, kvc, :], in_=ps,
                                     func=AF.Exp, scale=0.125,
                                     bias=shift_t[:, 0:1])
            if prev is not None:
                emit_av(*prev)
            prev = (h, attnT)
        emit_av(*prev)
        es_at.close()
        es_v.close()
        es_qt.close()

        # ---- Phase F: AO projection (fp8 DR) + residual ----
        x1 = p_x1.tile([P, TILES, D], f32, tag="x1")
        x2_re = x2_d.ap().rearrange("(t p) d -> p t d", p=P)
        for t in range(TILES):
            ps = pA()
            for half, sl in ((0, slice(0, 512)), (1, slice(512, 768))):
                for j in range(DC // 2):
                    nc.tensor.matmul(
                        ps[:, sl],
                        lhsT=aoutT[:, 2 * j:2 * j + 2, t * P:(t + 1) * P],
                        rhs=aow8[:, 2 * j:2 * j + 2, sl],
                        start=(j == 0), stop=(j == DC // 2 - 1),
                        perf_mode=DR)
            xr = xload.tile([P, D], f32, tag="xt")
            (nc.sync if t % 2 == 0 else nc.scalar).dma_start(xr, x2_re[:, t, :])
            nc.vector.tensor_tensor(out=x1[:, t, :], in0=ps[:, :768], in1=xr,
                                    op=OP.add)
        es_ao.close()

        # ---- Phase G: LN2 -> n2T bf16 (g2/b2 folded into up weights) ----
        n2T = p_n2.tile([P, DC, S], f8, tag="n2T")
        for t in range(TILES):
            z2 = work.tile([P, D], bf16, tag="zt")
            layernorm_z(x1[:, t, :], z2)
            for g in range(2):
                ps = psT.tile([P, 3 * P], bf16, tag="psT", name="psT_t")
                for j in range(3):
                    c = g * 3 + j
                    nc.tensor.transpose(ps[:, j * P:(j + 1) * P],
                                        z2[:, c * P:(c + 1) * P], id_b)
                nc.scalar.activation(
                    out=n2T[:, g * 3:(g + 1) * 3, t * P:(t + 1) * P],
                    in_=ps.rearrange("p (j q) -> p j q", j=3), func=AF.Copy)
            if flags["downb"]:
                nc.gpsimd.tensor_tensor(out=x1[:, t, :], in0=x1[:, t, :],
                                        in1=downb_sb, op=OP.add)

        # ---- Phase H: FFN ----
        y_re = y_d.ap().rearrange("(t p) d -> p t d", p=P)
        QTR = 256
        for q4 in range(S // QTR):
            pd = [pA() for _ in range(2)]

            def emit_down(hs, m2):
                for mi in range(2):
                    m = 2 * m2 + mi
                    for th in range(2):
                        for half, sl in ((0, slice(0, 512)),
                                         (1, slice(512, 768))):
                            nc.tensor.matmul(
                                pd[th][:, sl],
                                lhsT=hs[:, mi * QTR + th * P:
                                        mi * QTR + (th + 1) * P],
                                rhs=downw[:, m, sl],
                                start=(m == 0), stop=(m == FC - 1))

            pending = None
            for m2 in range(FC // 2):
                psu = pB()
                for mi in range(2):
                    m = 2 * m2 + mi
                    for j in range(DC // 2):
                        nc.tensor.matmul(
                            psu[:, mi * QTR:(mi + 1) * QTR],
                            lhsT=upw[:, 2 * j:2 * j + 2, m * P:(m + 1) * P],
                            rhs=n2T[:, 2 * j:2 * j + 2,
                                    q4 * QTR:(q4 + 1) * QTR],
                            start=(j == 0), stop=(j == DC // 2 - 1),
                            perf_mode=DR)
                hs = work.tile([P, 2 * QTR], bf16, tag="hstrip")
                if flags["upb"]:
                    hv = hs.rearrange("p (a q) -> p a q", a=2)
                    for mi in range(2):
                        nc.scalar.activation(
                            out=hv[:, mi, :],
                            in_=psu[:, mi * QTR:(mi + 1) * QTR],
                            func=AF.Gelu,
                            bias=upb_sb[:, 2 * m2 + mi:2 * m2 + mi + 1])
                else:
                    nc.scalar.activation(out=hs, in_=psu, func=AF.Gelu)
                if pending is not None:
                    emit_down(*pending)
                pending = (hs, m2)
            emit_down(*pending)
            for th in range(2):
                t = q4 * 2 + th
                ot = xload.tile([P, D], f32, tag="xt")
                nc.vector.tensor_tensor(out=ot, in0=pd[th][:, :768],
                                        in1=x1[:, t, :], op=OP.add)
                nc.sync.dma_start(y_re[:, t, :], ot)

        es_x1.close()
        es_up.close()
        es_dn.close()
        es_n2.close()

    nc.compile()
    return nc


def _get_nc(flags=None):
    if "nc" not in _CACHE:
        _CACHE["nc"] = _build(flags if flags is not None else
                              dict(qb=False, kb=False, finb=False, cb1=False,
                                   voutb=False, upb=False, downb=False))
    return _CACHE["nc"]


def _make_runner(flags):
    """Cached PJRT executor for the SPMD bass kernel (8 cores)."""
    import jax
    import concourse.mybir as mybir
    from concourse import bass2jax
    from jax.experimental.shard_map import shard_map
    from jax.sharding import Mesh, PartitionSpec

    nc = _get_nc(flags)
    bass2jax.install_neuronx_cc_hook()

    partition_name = (nc.partition_id_tensor.name
                      if nc.partition_id_tensor else None)
    in_names, out_names, out_avals, zero_outs = [], [], [], []
    for alloc in nc.m.functions[0].allocations:
        if not isinstance(alloc, mybir.MemoryLocationSet):
            continue
        name = alloc.memorylocations[0].name
        if alloc.kind == "ExternalInput":
            if name != partition_name:
                in_names.append(name)
        elif alloc.kind == "ExternalOutput":
            shape = tuple(alloc.tensor_shape)
            dtype = mybir.dt.np(alloc.dtype)
            out_names.append(name)
            out_avals.append(jax.core.ShapedArray(shape, dtype))
            zero_outs.append(np.zeros((NCORES * shape[0], *shape[1:]), dtype))
    n_params = len(in_names)
    n_outs = len(out_avals)
    all_in_names = list(in_names) + list(out_names)
    if partition_name is not None:
        all_in_names.append(partition_name)
    donate = tuple(range(n_params, n_params + n_outs))

    def _body(*args):
        operands = list(args)
        if partition_name is not None:
            operands.append(bass2jax.partition_id_tensor())
        outs = bass2jax._bass_exec_p.bind(
            *operands,
            out_avals=tuple(out_avals),
            in_names=tuple(all_in_names),
            out_names=tuple(out_names),
            lowering_input_output_aliases=(),
            sim_require_finite=True,
            sim_require_nnan=True,
            nc=nc,
        )
        return tuple(outs)

    devices = jax.devices()[:NCORES]
    mesh = Mesh(np.asarray(devices), ("core",))
    in_specs = (PartitionSpec("core"),) * (n_params + n_outs)
    out_specs = (PartitionSpec("core"),) * n_outs
    sharded = jax.jit(
        shard_map(_body, mesh=mesh, in_specs=in_specs, out_specs=out_specs,
                  check_rep=False),
        donate_argnums=donate, keep_unused=True)

    def run(in_maps, timing_iters=0):
        concat_in = [
            np.concatenate([np.asarray(in_maps[c][n]) for c in range(NCORES)],
                           axis=0)
            for n in in_names
        ]
        zeros = [z.copy() for z in zero_outs]
        out = sharded(*concat_in, *zeros)
        jax.block_until_ready(out)
        results = [np.asarray(o) for o in out]
        if timing_iters:
            import time
            from jax.sharding import NamedSharding
            dev_in = [jax.device_put(a, NamedSharding(mesh, PartitionSpec("core")))
                      for a in concat_in]
            times = []
            for _ in range(timing_iters):
                zs = [jax.device_put(z, NamedSharding(mesh, PartitionSpec("core")))
                      for z in zero_outs]
                jax.block_until_ready(zs)
                t0 = time.perf_counter()
                o = sharded(*dev_in, *zs)
                jax.block_until_ready(o)
                times.append(time.perf_counter() - t0)
            _CACHE["times"] = times
        return {name: results[i] for i, name in enumerate(out_names)}

    return run


def kernel(**inputs) -> np.ndarray:
    sig = tuple(
        np.asarray(inputs[k]).tobytes()[:64]
        for k in ("q_w", "up_w", "recipe", "n1_g"))
    if _CACHE.get("prep_sig") != sig:
        _CACHE["prep"] = _prep(inputs)
        _CACHE["prep_sig"] = sig
    w, flags, aob = _CACHE["prep"]
    if "runner" not in _CACHE:
        _CACHE["runner"] = _make_runner(flags)
    run = _CACHE["runner"]
    x = np.ascontiguousarray(np.asarray(inputs["x"], dtype=np.float32))
    in_maps = []
    for b in range(B):
        m = dict(w)
        m["x"] = np.ascontiguousarray(x[b])
        m["x2"] = np.ascontiguousarray(x[b] + aob)
        in_maps.append(m)
    out = run(in_maps, timing_iters=int(os.environ.get("KTIME", "0")))
    return out["y"].reshape(NCORES, S, D)


# revision 31
# speedup vs baseline: 1.0404x; 1.0404x over previous
"""Trainium2 Bass kernel for nn_DAWNLayer (moe_routing).

Strategy: data-parallel over batch B=8 across the 8 NeuronCores; each core
runs the full layer for one [S=1024, D=768] slice. Weights are replicated
and pre-processed once on the host (cached):
  * LN gains g1/g2 folded into the consumer weight matrices (q_w, k_w,
    basis_A, neuron table, up_w); zero biases compiled out.
  * Weights pre-transposed into SBUF layout and pre-cast: fp8 e4m3 for
    Q/K/AO projections, basis_A and the neuron table (DoubleRow matmuls),
    bf16 for the FFN and V-out.
  * neuron table = (softmax(recipe) * sigmoid(ctx_pat.sum/S)) @ basis_emb,
    exploiting that attn.mean(-1) of a softmax is the constant 1/S.

Device-side structure (per core):
  A: LN1 -> z (bf16) -> PE-transpose -> norm8 (fp8, d-major)
  B: routing scores (fp8 DR matmul), top-8 masked softmax, maskT,
     token_recipe
  C: basis projection (fp8 DR) * token_recipe -> v_sem -> V8 (fp8, with an
     appended ones column per head)
  D: Q/K projections (fp8 DR) -> QT/KT (fp8, head-major)
  E: attention without transposes: scoresT[kv,q] = KT-stationary matmul,
     exp -> fp8 attnT (unnormalized, shift -1.5), AV via fp8 DoubleRow with
     the ones column yielding the softmax denominator; normalize after AV.
  F: AO projection (fp8 DR) + residual (ao_b folded into x2 on host)
  G: LN2 -> n2T bf16 (g2/b2 folded into up weights)
  H: FFN (bf16), residual, writeback.
"""

import os
import numpy as np

B, S, D = 8, 1024, 768
H, DH = 12, 64
DFF = 3072
NN, NB, RK, TOPK = 96, 32, 64, 8
P = 128
TILES = S // P          # 8 token tiles
DC = D // P             # 6 chunks of d-model
FC = DFF // P           # 24 chunks of d_ff
NCORES = 8
HW = 65                 # head width in V (DH + ones column)
HPAD = 784              # TILES stride in V8 (pair-dim stride must be %16==0)

_CACHE = {}


def _f8(x):
    import ml_dtypes
    return np.clip(np.asarray(x, np.float32), -240.0, 240.0).astype(
        ml_dtypes.float8_e4m3)


def _bf(x):
    import ml_dtypes
    return np.asarray(x, np.float32).astype(ml_dtypes.bfloat16)


def _pon(w):
    """[D_in, N] -> [P, DC_in, N]  ((o p) n -> p o n)."""
    din, n = w.shape
    return np.ascontiguousarray(
        w.reshape(din // P, P, n).transpose(1, 0, 2))


def _prep(inputs):
    """Host-side one-time weight prep -> dict of DMA-ready arrays + flags."""
    f32 = np.float32
    g1 = np.asarray(inputs["n1_g"], f32)
    b1 = np.asarray(inputs["n1_b"], f32)
    g2 = np.asarray(inputs["n2_g"], f32)
    b2 = np.asarray(inputs["n2_b"], f32)
    q_w = np.asarray(inputs["q_w"], f32)
    k_w = np.asarray(inputs["k_w"], f32)
    ao_w = np.asarray(inputs["ao_w"], f32)
    up_w = np.asarray(inputs["up_w"], f32)
    down_w = np.asarray(inputs["down_w"], f32)
    basis_A = np.asarray(inputs["basis_A"], f32)
    vout_w = np.asarray(inputs["vout_w"], f32)
    recipe = np.asarray(inputs["recipe"], f32)
    ctx = np.asarray(inputs["ctx_pat"], f32)
    bemb = np.asarray(inputs["basis_emb"], f32)

    r = recipe - recipe.max(-1, keepdims=True)
    e = np.exp(r)
    recipe_norm = e / e.sum(-1, keepdims=True)                  # [NN, NB]
    sig = 1.0 / (1.0 + np.exp(-ctx.sum(-1) / S))                # [NN]
    nemb_s = (recipe_norm * sig[:, None]) @ bemb                # [NN, D]

    w = {}
    w["qw8"] = _f8(_pon(q_w * g1[:, None]))                     # [P,DC,D]
    w["kw8"] = _f8(_pon(k_w * g1[:, None]))
    w["aow8"] = _f8(_pon(ao_w))
    # basis8[p, o, n*RK + r] = basis_A[n, o*P+p, r] * g1
    bA = (basis_A * g1[None, :, None]).transpose(1, 0, 2).reshape(D, NB * RK)
    w["basis8"] = _f8(_pon(bA))                                 # [P,DC,NB*RK]
    w["upw"] = _f8(_pon(up_w * g2[:, None]))                    # [P,DC,DFF]
    w["downw"] = _bf(_pon(down_w))                              # [P,FC,D]
    w["voutw"] = _bf(vout_w)                                    # [RK,D]
    # nembT[p, o, n] = nemb_s[n, o*P+p] * g1[o*P+p]
    w["nembT"] = _f8(_pon((nemb_s * g1[None, :]).T))            # [P,DC,NN]
    w["recipe_norm"] = _bf(recipe_norm)                         # [NN,NB]

    flags = {}
    qb = np.asarray(inputs["q_b"], f32) + b1 @ q_w
    kb = np.asarray(inputs["k_b"], f32) + b1 @ k_w
    flags["qb"] = bool(np.any(qb)); flags["kb"] = bool(np.any(kb))
    if flags["qb"]:
        w["qb"] = np.ascontiguousarray(qb.reshape(DC, P).T).astype(f32)
    if flags["kb"]:
        w["kb"] = np.ascontiguousarray(kb.reshape(DC, P).T).astype(f32)
    finb = b1 @ nemb_s.T                                        # [NN]
    flags["finb"] = bool(np.any(finb))
    if flags["finb"]:
        w["finb"] = np.ascontiguousarray(
            np.broadcast_to(finb, (P, NN))).astype(f32)
    cb1 = np.einsum("d,ndr->nr", b1, basis_A).reshape(NB * RK)
    flags["cb1"] = bool(np.any(cb1))
    if flags["cb1"]:
        w["cb1"] = _f8(cb1.reshape(1, NB * RK))
    vob = np.asarray(inputs["vout_b"], f32)
    flags["voutb"] = bool(np.any(vob))
    if flags["voutb"]:
        w["voutb"] = np.ascontiguousarray(np.broadcast_to(vob, (P, D))).astype(f32)
    upb = np.asarray(inputs["up_b"], f32) + b2 @ up_w
    flags["upb"] = bool(np.any(upb))
    if flags["upb"]:
        w["upb"] = np.ascontiguousarray(upb.reshape(FC, P).T).astype(f32)
    dnb = np.asarray(inputs["down_b"], f32)
    flags["downb"] = bool(np.any(dnb))
    if flags["downb"]:
        w["downb"] = np.ascontiguousarray(np.broadcast_to(dnb, (P, D))).astype(f32)
    aob = np.asarray(inputs["ao_b"], f32)
    return w, flags, aob


def _build(flags):
    import concourse.bass as bass
    import concourse.bacc as bacc
    import concourse.mybir as mybir
    from concourse.tile import TileContext
    from concourse.masks import make_identity
    from contextlib import ExitStack

    f32 = mybir.dt.float32
    bf16 = mybir.dt.bfloat16
    f8 = mybir.dt.float8e4
    AF = mybir.ActivationFunctionType
    OP = mybir.AluOpType
    AX = mybir.AxisListType
    DR = mybir.MatmulPerfMode.DoubleRow

    nc = bacc.Bacc("TRN2", target_bir_lowering=False, debug=False,
                   num_devices=NCORES)

    d_in = {}
    def din(name, shape, dt):
        d_in[name] = nc.dram_tensor(name, list(shape), dt, kind="ExternalInput")
        return d_in[name]

    x_d = din("x", (S, D), f32)
    x2_d = din("x2", (S, D), f32)
    qw8_d = din("qw8", (P, DC, D), f8)
    kw8_d = din("kw8", (P, DC, D), f8)
    aow8_d = din("aow8", (P, DC, D), f8)
    basis8_d = din("basis8", (P, DC, NB * RK), f8)
    upw_d = din("upw", (P, DC, DFF), f8)
    downw_d = din("downw", (P, FC, D), bf16)
    voutw_d = din("voutw", (RK, D), bf16)
    nembT_d = din("nembT", (P, DC, NN), f8)
    recipe_d = din("recipe_norm", (NN, NB), bf16)
    if flags["qb"]:
        qb_d = din("qb", (P, DC), f32)
    if flags["kb"]:
        kb_d = din("kb", (P, DC), f32)
    if flags["finb"]:
        finb_d = din("finb", (P, NN), f32)
    if flags["cb1"]:
        cb1_d = din("cb1", (1, NB * RK), f8)
    if flags["voutb"]:
        voutb_d = din("voutb", (P, D), f32)
    if flags["upb"]:
        upb_d = din("upb", (P, FC), f32)
    if flags["downb"]:
        downb_d = din("downb", (P, D), f32)
    y_d = nc.dram_tensor("y", [S, D], f32, kind="ExternalOutput")

    with TileContext(nc, pool_alloc_mode="queue") as tc, ExitStack() as top, \
         nc.allow_low_precision(reason="bf16/fp8 pipeline, fp32 accum"):
        psA = top.enter_context(tc.tile_pool(name="psA", bufs=2, space="PSUM"))
        psB = top.enter_context(tc.tile_pool(name="psB", bufs=2, space="PSUM"))
        psT = top.enter_context(tc.tile_pool(name="psT", bufs=2, space="PSUM"))
        singles = top.enter_context(tc.tile_pool(name="singles", side="left", bufs=1))
        work = top.enter_context(tc.tile_pool(name="work", side="left", bufs=3))
        xload = top.enter_context(tc.tile_pool(name="xload", side="left", bufs=4))

        # left-side pools, bottom (longest-lived) to top (shortest-lived)
        es_dn = ExitStack()
        p_dn = es_dn.enter_context(tc.tile_pool(name="p_dn", side="left", bufs=1))
        es_up = ExitStack()
        p_up = es_up.enter_context(tc.tile_pool(name="p_up", side="left", bufs=1))
        es_x1 = ExitStack()
        p_x1 = es_x1.enter_context(tc.tile_pool(name="p_x1", side="left", bufs=1))
        es_ao = ExitStack()
        p_ao = es_ao.enter_context(tc.tile_pool(name="p_ao", side="left", bufs=1))
        es_v = ExitStack()
        p_v = es_v.enter_context(tc.tile_pool(name="p_v", side="left", bufs=1))
        es_at = ExitStack()
        p_at = es_at.enter_context(tc.tile_pool(name="p_at", side="left", bufs=2))
        es_n1 = ExitStack()
        p_n1 = es_n1.enter_context(tc.tile_pool(name="p_n1", side="left", bufs=1))
        es_b = ExitStack()
        p_b = es_b.enter_context(tc.tile_pool(name="p_b", side="left", bufs=1))
        # right-side pools
        es_n2 = ExitStack()
        p_n2 = es_n2.enter_context(tc.tile_pool(name="p_n2", side="right", bufs=1))
        es_qt = ExitStack()
        p_qt = es_qt.enter_context(tc.tile_pool(name="p_qt", side="right", bufs=1))
        es_w1 = ExitStack()
        p_w1 = es_w1.enter_context(tc.tile_pool(name="p_w1", side="right", bufs=1))

        def pA():  return psA.tile([P, 1024], f32, tag="psA", name="psA_t")
        def pB():  return psB.tile([P, 512], f32, tag="psB", name="psB_t")

        # ---- constants ----
        id_b = singles.tile([P, P], bf16)
        make_identity(nc, id_b)
        eps_t = singles.tile([P, 1], f32)
        nc.vector.memset(eps_t, 1e-5)
        shift_t = singles.tile([P, 1], f32)
        nc.vector.memset(shift_t, -1.5)
        ones_bf = singles.tile([1, RK], bf16)
        nc.vector.memset(ones_bf, 1.0)

        # ---- weights (straight DMA, pre-laid-out) ----
        nembT = singles.tile([P, DC, NN], f8)
        nc.scalar.dma_start(nembT, nembT_d.ap())
        recipe_sb = singles.tile([NN, NB], bf16)
        nc.scalar.dma_start(recipe_sb, recipe_d.ap())
        basis8 = p_w1.tile([P, DC, NB * RK], f8)
        nc.scalar.dma_start(basis8, basis8_d.ap())
        qw8 = p_w1.tile([P, DC, D], f8)
        nc.scalar.dma_start(qw8, qw8_d.ap())
        kw8 = p_w1.tile([P, DC, D], f8)
        nc.scalar.dma_start(kw8, kw8_d.ap())
        voutw = singles.tile([RK, D], bf16)
        nc.scalar.dma_start(voutw, voutw_d.ap())
        aow8 = singles.tile([P, DC, D], f8)
        nc.gpsimd.dma_start(aow8, aow8_d.ap())
        if flags["qb"]:
            qb_sb = singles.tile([P, DC], f32)
            nc.scalar.dma_start(qb_sb, qb_d.ap())
        if flags["kb"]:
            kb_sb = singles.tile([P, DC], f32)
            nc.scalar.dma_start(kb_sb, kb_d.ap())
        if flags["finb"]:
            finb_sb = singles.tile([P, NN], f32)
            nc.scalar.dma_start(finb_sb, finb_d.ap())
        if flags["cb1"]:
            cb1_sb = singles.tile([1, NB * RK], f8)
            nc.scalar.dma_start(cb1_sb, cb1_d.ap())
            ones18 = singles.tile([1, P], f8)
            nc.vector.memset(ones18, 1.0)
        if flags["voutb"]:
            voutb_sb = singles.tile([P, D], f32)
            nc.scalar.dma_start(voutb_sb, voutb_d.ap())
        if flags["upb"]:
            upb_sb = singles.tile([P, FC], f32)
            nc.scalar.dma_start(upb_sb, upb_d.ap())
        if flags["downb"]:
            downb_sb = singles.tile([P, D], f32)
            nc.scalar.dma_start(downb_sb, downb_d.ap())

        # ---- Phase A: LN1 -> norm8 (fp8, d-major) ----
        norm8 = p_n1.tile([P, DC, S], f8, tag="norm8")

        def layernorm_z(xt, out_z):
            """z = (x - mean) * rstd, token-major; out_z bf16 [P, D]."""
            stats = work.tile([P, 3, 6], f32, tag="ln_stats")
            xv = xt.rearrange("p (a q) -> p a q", a=3)
            for a in range(3):
                nc.vector.bn_stats(out=stats[:, a, :], in_=xv[:, a, :])
            mv = work.tile([P, 2], f32, tag="ln_mv")
            nc.vector.bn_aggr(out=mv, in_=stats)
            std = work.tile([P, 1], f32, tag="ln_std")
            nc.scalar.activation(out=std, in_=mv[:, 1:2], func=AF.Sqrt,
                                 bias=eps_t[:, 0:1])
            rstd = work.tile([P, 1], f32, tag="ln_rstd")
            nc.vector.reciprocal(rstd, std)
            nc.vector.tensor_scalar(out=out_z, in0=xt, scalar1=mv[:, 0:1],
                                    scalar2=rstd[:, 0:1], op0=OP.subtract,
                                    op1=OP.mult)

        x_re = x_d.ap().rearrange("(t p) d -> p t d", p=P)
        for t in range(TILES):
            xt = xload.tile([P, D], f32, tag="xt")
            nc.sync.dma_start(xt, x_re[:, t, :])
            z = work.tile([P, D], bf16, tag="zt")
            layernorm_z(xt, z)
            ps = psT.tile([P, D], bf16, tag="psT", name="psT_t")
            for c in range(DC):
                nc.tensor.transpose(ps[:, c * P:(c + 1) * P],
                                    z[:, c * P:(c + 1) * P], id_b)
            nc.scalar.activation(
                out=norm8[:, :, t * P:(t + 1) * P],
                in_=ps.rearrange("p (j q) -> p j q", j=DC), func=AF.Copy)

        # ---- Phase B: routing scores -> maskT, token_recipe ----
        maskT = p_b.tile([NN, S], bf16)
        tr_sb = singles.tile([P, TILES, NB], bf16)
        for t in range(TILES):
            ps = psT.tile([P, NN], f32, tag="psT", name="psT_t")
            for j in range(DC // 2):
                nc.tensor.matmul(ps[:P, :NN],
                                 lhsT=norm8[:, 2 * j:2 * j + 2,
                                            t * P:(t + 1) * P],
                                 rhs=nembT[:, 2 * j:2 * j + 2, :],
                                 start=(j == 0), stop=(j == DC // 2 - 1),
                                 perf_mode=DR)
            fin = work.tile([P, NN], f32, tag="fin")
            if flags["finb"]:
                nc.vector.tensor_tensor(out=fin, in0=ps[:P, :NN],
                                        in1=finb_sb, op=OP.add)
            else:
                nc.vector.tensor_copy(out=fin, in_=ps[:P, :NN])
            mx = work.tile([P, 8], f32, tag="mx")
            nc.vector.max(out=mx, in_=fin)
            nmx = work.tile([P, 1], f32, tag="nmx")
            nc.vector.tensor_scalar_mul(nmx, mx[:, 0:1], -1.0)
            e = work.tile([P, NN], bf16, tag="e")
            nc.scalar.activation(out=e, in_=fin, func=AF.Exp, bias=nmx[:, 0:1])
            msk = work.tile([P, NN], f32, tag="msk")
            nc.vector.tensor_scalar(out=msk, in0=fin, scalar1=mx[:, 7:8],
                                    scalar2=None, op0=OP.is_ge)
            nc.vector.tensor_tensor(out=e, in0=e, in1=msk, op=OP.mult)
            den = work.tile([P, 1], f32, tag="den")
            nc.vector.tensor_reduce(out=den, in_=e, axis=AX.X, op=OP.add)
            idn = work.tile([P, 1], f32, tag="idn")
            nc.vector.reciprocal(idn, den)
            nc.vector.tensor_scalar_mul(e, e, idn[:, 0:1])
            ps2 = psT.tile([P, P], bf16, tag="psT", name="psT_t")
            nc.tensor.transpose(ps2[:NN, :P], e[:, :NN], id_b)
            nc.vector.tensor_copy(out=maskT[:, t * P:(t + 1) * P],
                                  in_=ps2[:NN, :P])
            ps3 = psT.tile([P, NB], f32, tag="psT", name="psT_t")
            nc.tensor.matmul(ps3[:P, :NB], lhsT=maskT[:, t * P:(t + 1) * P],
                             rhs=recipe_sb, start=True, stop=True)
            nc.vector.tensor_copy(out=tr_sb[:, t, :], in_=ps3[:P, :NB])

        # ---- Phase C: basis projection (fp8 DR) -> v_sem -> V8 ----
        vsemT = singles.tile([RK, TILES, P], bf16)
        NGRP = 4
        GN = NB // NGRP     # 8 basis entries per 512-wide group
        for t in range(TILES):
            scf = work.tile([P, NB, RK], bf16, tag="scf")
            for g in range(NGRP):
                ps = pB()
                first = True
                if flags["cb1"]:
                    nc.tensor.matmul(
                        ps[:, :512], lhsT=ones18,
                        rhs=cb1_sb[:, g * 512:(g + 1) * 512],
                        start=True, stop=False, skip_group_check=True)
                    first = False
                for j in range(DC // 2):
                    nc.tensor.matmul(
                        ps[:, :512],
                        lhsT=norm8[:, 2 * j:2 * j + 2, t * P:(t + 1) * P],
                        rhs=basis8[:, 2 * j:2 * j + 2, g * 512:(g + 1) * 512],
                        start=first and (j == 0), stop=(j == DC // 2 - 1),
                        perf_mode=DR)
                sc = scf[:, g * GN:(g + 1) * GN, :]
                nc.scalar.activation(out=sc, in_=ps.rearrange(
                    "p (n r) -> p n r", n=GN), func=AF.Copy)
                nc.vector.tensor_tensor(
                    out=sc, in0=sc,
                    in1=tr_sb[:, t, g * GN:(g + 1) * GN, None].to_broadcast(
                        [P, GN, RK]),
                    op=OP.mult)
            nc.vector.tensor_tensor(out=scf[:, 0:16, :], in0=scf[:, 0:16, :],
                                    in1=scf[:, 16:32, :], op=OP.add)
            nc.vector.tensor_tensor(out=scf[:, 0:8, :], in0=scf[:, 0:8, :],
                                    in1=scf[:, 8:16, :], op=OP.add)
            nc.vector.tensor_tensor(out=scf[:, 0:4, :], in0=scf[:, 0:4, :],
                                    in1=scf[:, 4:8, :], op=OP.add)
            nc.vector.tensor_tensor(out=scf[:, 0:2, :], in0=scf[:, 0:2, :],
                                    in1=scf[:, 2:4, :], op=OP.add)
            vsem = work.tile([P, RK], bf16, tag="vsem")
            nc.vector.tensor_tensor(out=vsem, in0=scf[:, 0, :],
                                    in1=scf[:, 1, :], op=OP.add)
            ps2 = psT.tile([P, P], bf16, tag="psT", name="psT_t")
            nc.tensor.transpose(ps2[:RK, :P], vsem, id_b)
            nc.vector.tensor_copy(out=vsemT[:, t, :], in_=ps2[:RK, :P])

        # ---- V build: V8 [P, TILES, H*HW] fp8 with ones columns ----
        V8 = p_v.tile([P, TILES, HPAD], f8, tag="V8")
        nc.vector.memset(V8, 1.0)
        for t in range(TILES):
            ps = pA()
            nc.tensor.matmul(ps[:, 0:512], lhsT=vsemT[:, t, :],
                             rhs=voutw[:, 0:512], start=True, stop=True)
            nc.tensor.matmul(ps[:, 512:768], lhsT=vsemT[:, t, :],
                             rhs=voutw[:, 512:768], start=True, stop=True)
            dst = V8[:, t, 0:H * HW].rearrange("p (h e) -> p h e", h=H)[:, :, 0:DH]
            src = ps[:, :768].rearrange("p (h e) -> p h e", h=H)
            if flags["voutb"]:
                nc.vector.tensor_tensor(
                    out=dst, in0=src,
                    in1=voutb_sb.rearrange("p (h e) -> p h e", h=H),
                    op=OP.add)
            else:
                nc.vector.tensor_copy(out=dst, in_=src)

        # ---- Phase D: Q/K projections (fp8 DR) -> QT/KT fp8 ----
        QT = p_qt.tile([P, DC, S], f8, tag="QT")
        KT = p_qt.tile([P, DC, S], f8, tag="KT")
        for (w8, bflag, bias_sb, out_t) in (
                (qw8, flags["qb"], (qb_sb if flags["qb"] else None), QT),
                (kw8, flags["kb"], (kb_sb if flags["kb"] else None), KT)):
            for m in range(DC):
                for half in range(2):
                    sl = slice(half * 512, (half + 1) * 512)
                    ps = pB()
                    for j in range(DC // 2):
                        nc.tensor.matmul(
                            ps[:, :512],
                            lhsT=w8[:, 2 * j:2 * j + 2, m * P:(m + 1) * P],
                            rhs=norm8[:, 2 * j:2 * j + 2, sl],
                            start=(j == 0), stop=(j == DC // 2 - 1),
                            perf_mode=DR)
                    if bflag:
                        nc.scalar.activation(out=out_t[:, m, sl], in_=ps,
                                             func=AF.Identity,
                                             bias=bias_sb[:, m:m + 1])
                    else:
                        nc.scalar.activation(out=out_t[:, m, sl], in_=ps,
                                             func=AF.Copy)


        es_b.close()
        es_n1.close()
        es_w1.close()

        # ---- FFN weight prefetch (overlaps attention) ----
        upw = p_up.tile([P, DC, DFF], f8, tag="upw")
        for c in range(DC):
            nc.scalar.dma_start(upw[:, c, :], upw_d.ap()[:, c, :])
        downw = p_dn.tile([P, FC, D], bf16, tag="downw")
        for c4 in range(FC // 4):
            nc.scalar.dma_start(downw[:, 4 * c4:4 * c4 + 4, :],
                                downw_d.ap()[:, 4 * c4:4 * c4 + 4, :])

        # ---- Phase E: attention (AV of head h-1 overlaps exp of head h) ----
        aoutT = p_ao.tile([P, DC, S], f8, tag="aoutT")

        def emit_av(h, attnT):
            hp = (h % 2) * DH
            hc = h // 2
            for qc in range(2):
                sl = slice(qc * 512, (qc + 1) * 512)
                psv = pB()
                for jt in range(TILES // 2):
                    nc.tensor.matmul(
                        psv[0:HW, :512],
                        lhsT=V8[:, 2 * jt:2 * jt + 2, h * HW:(h + 1) * HW],
                        rhs=attnT[:, 2 * jt:2 * jt + 2, sl],
                        start=(jt == 0), stop=(jt == TILES // 2 - 1),
                        perf_mode=DR)
                rden = work.tile([1, 512], bf16, tag="rden")
                nc.vector.reciprocal(rden, psv[DH:HW, :512])
                psd = psT.tile([RK, 512], f32, tag="psT", name="psT_t")
                nc.tensor.matmul(psd, lhsT=ones_bf, rhs=rden,
                                 start=True, stop=True)
                den64 = work.tile([RK, 512], bf16, tag="den64")
                nc.vector.tensor_copy(out=den64, in_=psd)
                nc.vector.tensor_tensor(
                    out=aoutT[hp:hp + DH, hc, sl], in0=psv[0:DH, :512],
                    in1=den64, op=OP.mult)

        prev = None
        for h in range(H):
            hp = (h % 2) * DH
            hc = h // 2
            attnT = p_at.tile([P, TILES, S], f8, tag="attnT")
            for kvc in range(TILES):
                ps = pA()
                for qh in range(2):
                    qsl = slice(qh * 512, (qh + 1) * 512)
                    nc.tensor.matmul(
                        ps[:, qsl],
                        lhsT=KT[hp:hp + DH, hc, kvc * P:(kvc + 1) * P],
                        rhs=QT[hp:hp + DH, hc, qsl],
                        start=True, stop=True)
                nc.scalar.activation(out=attnT[:, kvc, :], in_=ps,
                                     func=AF.Exp, scale=0.125,
                                     bias=shift_t[:, 0:1])
            if prev is not None:
                emit_av(*prev)
            prev = (h, attnT)
        emit_av(*prev)
        es_at.close()
        es_v.close()
        es_qt.close()

        # ---- Phase F: AO projection (fp8 DR) + residual ----
        x1 = p_x1.tile([P, TILES, D], f32, tag="x1")
        x2_re = x2_d.ap().rearrange("(t p) d -> p t d", p=P)
        for t in range(TILES):
            ps = pA()
            for half, sl in ((0, slice(0, 512)), (1, slice(512, 768))):
                for j in range(DC // 2):
                    nc.tensor.matmul(
                        ps[:, sl],
                        lhsT=aoutT[:, 2 * j:2 * j + 2, t * P:(t + 1) * P],
                        rhs=aow8[:, 2 * j:2 * j + 2, sl],
                        start=(j == 0), stop=(j == DC // 2 - 1),
                        perf_mode=DR)
            xr = xload.tile([P, D], f32, tag="xt")
            (nc.sync if t % 2 == 0 else nc.scalar).dma_start(xr, x2_re[:, t, :])
            nc.vector.tensor_tensor(out=x1[:, t, :], in0=ps[:, :768], in1=xr,
                                    op=OP.add)
        es_ao.close()

        # ---- Phase G: LN2 -> n2T bf16 (g2/b2 folded into up weights) ----
        n2T = p_n2.tile([P, DC, S], f8, tag="n2T")
        for t in range(TILES):
            z2 = work.tile([P, D], bf16, tag="zt")
            layernorm_z(x1[:, t, :], z2)
            ps = psT.tile([P, D], bf16, tag="psT", name="psT_t")
            for c in range(DC):
                nc.tensor.transpose(ps[:, c * P:(c + 1) * P],
                                    z2[:, c * P:(c + 1) * P], id_b)
            nc.scalar.activation(
                out=n2T[:, :, t * P:(t + 1) * P],
                in_=ps.rearrange("p (j q) -> p j q", j=DC), func=AF.Copy)
            if flags["downb"]:
                nc.gpsimd.tensor_tensor(out=x1[:, t, :], in0=x1[:, t, :],
                                        in1=downb_sb, op=OP.add)

        # ---- Phase H: FFN ----
        y_re = y_d.ap().rearrange("(t p) d -> p t d", p=P)
        QTR = 256
        for q4 in range(S // QTR):
            pd = [pA() for _ in range(2)]

            def emit_down(hs, m2):
                for mi in range(2):
                    m = 2 * m2 + mi
                    for th in range(2):
                        for half, sl in ((0, slice(0, 512)),
                                         (1, slice(512, 768))):
                            nc.tensor.matmul(
                                pd[th][:, sl],
                                lhsT=hs[:, mi * QTR + th * P:
                                        mi * QTR + (th + 1) * P],
                                rhs=downw[:, m, sl],
                                start=(m == 0), stop=(m == FC - 1))

            pending = None
            for m2 in range(FC // 2):
                psu = pB()
                for mi in range(2):
                    m = 2 * m2 + mi
                    for j in range(DC // 2):
                        nc.tensor.matmul(
                            psu[:, mi * QTR:(mi + 1) * QTR],
                            lhsT=upw[:, 2 * j:2 * j + 2, m * P:(m + 1) * P],
                            rhs=n2T[:, 2 * j:2 * j + 2,
                                    q4 * QTR:(q4 + 1) * QTR],
                            start=(j == 0), stop=(j == DC // 2 - 1),
                            perf_mode=DR)
                hs = work.tile([P, 2 * QTR], bf16, tag="hstrip")
                if flags["upb"]:
                    hv = hs.rearrange("p (a q) -> p a q", a=2)
                    for mi in range(2):
                        nc.scalar.activation(
                            out=hv[:, mi, :],
                            in_=psu[:, mi * QTR:(mi + 1) * QTR],
                            func=AF.Gelu,
                            bias=upb_sb[:, 2 * m2 + mi:2 * m2 + mi + 1])
                else:
                    nc.scalar.activation(out=hs, in_=psu, func=AF.Gelu)
                if pending is not None:
                    emit_down(*pending)
                pending = (hs, m2)
            emit_down(*pending)
            for th in range(2):
                t = q4 * 2 + th
                ot = xload.tile([P, D], f32, tag="xt")
                nc.vector.tensor_tensor(out=ot, in0=pd[th][:, :768],
                                        in1=x1[:, t, :], op=OP.add)
                nc.sync.dma_start(y_re[:, t, :], ot)

        es_x1.close()
        es_up.close()
        es_dn.close()
        es_n2.close()

    nc.compile()
    return nc


def _get_nc(flags=None):
    if "nc" not in _CACHE:
        _CACHE["nc"] = _build(flags if flags is not None else
                              dict(qb=False, kb=False, finb=False, cb1=False,
                                   voutb=False, upb=False, downb=False))
    return _CACHE["nc"]


def _make_runner(flags):
    """Cached PJRT executor for the SPMD bass kernel (8 cores)."""
    import jax
    import concourse.mybir as mybir
    from concourse import bass2jax
    from jax.experimental.shard_map import shard_map
    from jax.sharding import Mesh, PartitionSpec

    nc = _get_nc(flags)
    bass2jax.install_neuronx_cc_hook()

    partition_name = (nc.partition_id_tensor.name
                      if nc.partition_id_tensor else None)
    in_names, out_names, out_avals, zero_outs = [], [], [], []
    for alloc in nc.m.functions[0].allocations:
        if not isinstance(alloc, mybir.MemoryLocationSet):
            continue
        name = alloc.memorylocations[0].name
        if alloc.kind == "ExternalInput":
            if name != partition_name:
                in_names.append(name)
        elif alloc.kind == "ExternalOutput":
            shape = tuple(alloc.tensor_shape)
            dtype = mybir.dt.np(alloc.dtype)
            out_names.append(name)
            out_avals.append(jax.core.ShapedArray(shape, dtype))
            zero_outs.append(np.zeros((NCORES * shape[0], *shape[1:]), dtype))
    n_params = len(in_names)
    n_outs = len(out_avals)
    all_in_names = list(in_names) + list(out_names)
    if partition_name is not None:
        all_in_names.append(partition_name)
    donate = tuple(range(n_params, n_params + n_outs))

    def _body(*args):
        operands = list(args)
        if partition_name is not None:
            operands.append(bass2jax.partition_id_tensor())
        outs = bass2jax._bass_exec_p.bind(
            *operands,
            out_avals=tuple(out_avals),
            in_names=tuple(all_in_names),
            out_names=tuple(out_names),
            lowering_input_output_aliases=(),
            sim_require_finite=True,
            sim_require_nnan=True,
            nc=nc,
        )
        return tuple(outs)

    devices = jax.devices()[:NCORES]
    mesh = Mesh(np.asarray(devices), ("core",))
    in_specs = (PartitionSpec("core"),) * (n_params + n_outs)
    out_specs = (PartitionSpec("core"),) * n_outs
    sharded = jax.jit(
        shard_map(_body, mesh=mesh, in_specs=in_specs, out_specs=out_specs,
                  check_rep=False),
        donate_argnums=donate, keep_unused=True)

    def run(in_maps, timing_iters=0):
        concat_in = [
            np.concatenate([np.asarray(in_maps[c][n]) for c in range(NCORES)],
                           axis=0)
            for n in in_names
        ]
        zeros = [z.copy() for z in zero_outs]
        out = sharded(*concat_in, *zeros)
        jax.block_until_ready(out)
        results = [np.asarray(o) for o in out]
        if timing_iters:
            import time
            from jax.sharding import NamedSharding
            dev_in = [jax.device_put(a, NamedSharding(mesh, PartitionSpec("core")))
                      for a in concat_in]
            times = []
            for _ in range(timing_iters):
                zs = [jax.device_put(z, NamedSharding(mesh, PartitionSpec("core")))
                      for z in zero_outs]
                jax.block_until_ready(zs)
                t0 = time.perf_counter()
                o = sharded(*dev_in, *zs)
                jax.block_until_ready(o)
                times.append(time.perf_counter() - t0)
            _CACHE["times"] = times
        return {name: results[i] for i, name in enumerate(out_names)}

    return run


def kernel(**inputs) -> np.ndarray:
    sig = tuple(
        np.asarray(inputs[k]).tobytes()[:64]
        for k in ("q_w", "up_w", "recipe", "n1_g"))
    if _CACHE.get("prep_sig") != sig:
        _CACHE["prep"] = _prep(inputs)
        _CACHE["prep_sig"] = sig
    w, flags, aob = _CACHE["prep"]
    if "runner" not in _CACHE:
        _CACHE["runner"] = _make_runner(flags)
    run = _CACHE["runner"]
    x = np.ascontiguousarray(np.asarray(inputs["x"], dtype=np.float32))
    in_maps = []
    for b in range(B):
        m = dict(w)
        m["x"] = np.ascontiguousarray(x[b])
        m["x2"] = np.ascontiguousarray(x[b] + aob)
        in_maps.append(m)
    out = run(in_maps, timing_iters=int(os.environ.get("KTIME", "0")))
    return out["y"].reshape(NCORES, S, D)
